# revision 17
# baseline (speedup 1.0000x reference)
"""TRN2 Bass kernel for nn_AttentionWrapper (GQA attention + RoPE + causal mask
+ post-softmax suppression), tensor-parallel over heads across 8 NeuronCores.

Sharding: core i owns q-heads 4i..4i+3 and kv-head i (GQA group i), i.e. rows
512i..512(i+1) of Wq, rows 128i..128(i+1) of Wk/Wv, and columns 512i..512(i+1)
of Wo. hidden_states is replicated; each core emits a full-shape partial of
the output projection and the host sums the 8 partials.

Per-core device program (identical SPMD program, per-core weight data):
  P1  QT/KT = W^T-major projections fused with RoPE (rotate-half via a signed
      permutation matmul), V via XT-stationary matmuls + PE transpose; spill
      QT/KT/V to internal DRAM.
  P2  Flash-style attention per (batch, head) in score-transposed layout
      ST[k,q]: exp on ScalarE (no max subtraction - scores are bounded),
      causal handled by variable-width matmuls + a triangular mask on the
      diagonal blocks, row sums via an all-ones stationary matmul (replicated
      across partitions so normalization broadcasts for free), suppression as
      a single-column multiply, OT = V^T P^T accumulated in PSUM.
  P3  Output projection from OT tiles against the Wo column shard; partial
      written as [B*S, HID] float32.

All matmul operands use dt.float32r (full-rate fp32 PE mode).
"""
import math

import numpy as np

import concourse.bass as bass
import concourse.bacc as bacc
import concourse.mybir as mybir
import concourse.tile as tile
from concourse.bass_utils import run_bass_kernel_spmd

B, S, HID = 2, 2048, 4096
NH, NKV, HD = 32, 8, 128
NCORES = 8
HQ = NH // NCORES            # 4 q heads per core
T = B * S                    # 4096 token axis (b*S + s)
THETA = 10000.0
SUPPRESS = 0.1
SCALE = 1.0 / math.sqrt(HD)

P = 128
TC = 512                     # P1 token chunk
NCH = T // TC                # 8
KS = 4                       # k subtiles per K tile (K_TILE = 512)
NKT = HID // (P * KS)        # 8 K tiles over HID
WC = 512 + HD + HD           # 768 fused wqkv columns (4 q heads, k, v)
QC = 512                     # P2 query chunk
NQC = S // QC                # 4 per batch
NKB = S // P                 # 16 key blocks per batch
MC = 512                     # P3 output column chunk
DT = mybir.dt.float32r
F32 = mybir.dt.float32


def _build_program(sup_plan, causal):
    """sup_plan: list over b of (qc_idx, tloc, sorted kb list).
    causal: per-batch bool — True applies the causal structure (variable-width
    score matmuls + triangular diag mask), False runs full attention."""
    nc = bacc.Bacc("TRN2", target_bir_lowering=False, debug=False)

    xt = nc.dram_tensor("xt", [HID, T], DT, kind="ExternalInput")
    wqkv = nc.dram_tensor("wqkv", [HID, WC], DT, kind="ExternalInput")
    wo = nc.dram_tensor("wo", [HQ * HD, HID], DT, kind="ExternalInput")
    cs = nc.dram_tensor("cs", [HD, T], DT, kind="ExternalInput")
    sn = nc.dram_tensor("sn", [HD, T], DT, kind="ExternalInput")
    rmat = nc.dram_tensor("rmat", [HD, HD], DT, kind="ExternalInput")
    tri = nc.dram_tensor("tri", [P, P], DT, kind="ExternalInput")
    ident = nc.dram_tensor("ident", [P, P], DT, kind="ExternalInput")
    ones = nc.dram_tensor("ones", [P, P], DT, kind="ExternalInput")
    sup = nc.dram_tensor("sup", [P, B, NKB], DT, kind="ExternalInput")
    out = nc.dram_tensor("out", [T, HID], F32, kind="ExternalOutput")

    qsp = nc.dram_tensor("qsp", [HQ, HD, T], DT)
    ksp = nc.dram_tensor("ksp", [HD, T], DT)
    vsp = nc.dram_tensor("vsp", [B, S // P, P, HD], DT)
    osp = nc.dram_tensor("osp", [B, HQ, HD, S], DT)

    xt4 = xt.rearrange("(kt ks p) t -> kt p ks t", p=P, ks=KS)
    wqkv4 = wqkv.rearrange("(kt ks p) c -> kt p ks c", p=P, ks=KS)
    wo3 = wo.rearrange("(jb p) m -> p jb m", p=P)

    with tile.TileContext(nc) as tc:
        # ---- constants used across phases -------------------------------
        with tc.tile_pool(name="const", bufs=1) as cpool:
            rmat_sb = cpool.tile([HD, HD], DT, tag="rmat")
            nc.sync.dma_start(rmat_sb[:], rmat[:])
            tri_sb = cpool.tile([P, P], DT, tag="tri")
            nc.sync.dma_start(tri_sb[:], tri[:])
            ident_sb = cpool.tile([P, P], DT, tag="ident")
            nc.sync.dma_start(ident_sb[:], ident[:])
            ones_sb = cpool.tile([P, P], DT, tag="ones")
            nc.sync.dma_start(ones_sb[:], ones[:])
            sup_sb = cpool.tile([P, B, NKB], DT, tag="sup")
            nc.sync.dma_start(sup_sb[:], sup[:])

            # ---- P1: projections + RoPE ---------------------------------
            with (
                tc.tile_pool(name="p1w", bufs=1) as wpool,
                tc.tile_pool(name="p1x", bufs=4) as xpool,
                tc.tile_pool(name="p1cs", bufs=2) as cspool,
                tc.tile_pool(name="p1raw", bufs=3) as rawpool,
                tc.tile_pool(name="p1tmp", bufs=3) as tmppool,
                tc.tile_pool(name="p1out", bufs=3) as opool,
                tc.tile_pool(name="p1ps", bufs=6, space="PSUM") as pspool,
                tc.tile_pool(name="p1rot", bufs=1, space="PSUM") as rotpool,
                tc.tile_pool(name="p1tp", bufs=1, space="PSUM") as tppool,
            ):
                wts = []
                for kt in range(NKT):
                    wt = wpool.tile([P, KS, WC], DT, tag=f"w{kt}")
                    nc.sync.dma_start(wt[:], wqkv4[kt])
                    wts.append(wt)

                for ch in range(NCH):
                    b = ch // (NCH // B)
                    cs_t = cspool.tile([HD, TC], DT, tag="cs")
                    nc.sync.dma_start(cs_t[:], cs[:, ch * TC:(ch + 1) * TC])
                    sn_t = cspool.tile([HD, TC], DT, tag="sn")
                    nc.sync.dma_start(sn_t[:], sn[:, ch * TC:(ch + 1) * TC])

                    # 6 live accumulators (one PSUM bank each); X streamed
                    # through a small rotating pool, one K-tile at a time.
                    pss = [pspool.tile([P, TC], F32, tag="acc", name=f"acc{ch}_{i}")
                           for i in range(6)]
                    for kt in range(NKT):
                        xtile = xpool.tile([P, KS, TC], DT, tag="x")
                        nc.sync.dma_start(
                            xtile[:], xt4[kt, :, :, ch * TC:(ch + 1) * TC])
                        for ks_ in range(KS):
                            for ob in range(6):
                                nc.tensor.matmul(
                                    pss[ob],
                                    wts[kt][:, ks_, ob * P:(ob + 1) * P],
                                    xtile[:, ks_],
                                    start=(kt == 0 and ks_ == 0),
                                    stop=(kt == NKT - 1 and ks_ == KS - 1))

                    for ob in range(6):
                        ps = pss[ob]
                        if ob < 5:
                            # q head 0..3 or k: RoPE then spill [d, t]
                            raw = rawpool.tile([P, TC], DT, tag="raw")
                            nc.scalar.copy(raw[:], ps[:])
                            rot = rotpool.tile([P, TC], F32)
                            nc.tensor.matmul(rot[:], rmat_sb[:], raw[:],
                                             start=True, stop=True)
                            t1 = tmppool.tile([P, TC], F32, tag="t1")
                            nc.vector.tensor_mul(t1[:], raw[:], cs_t[:])
                            t2 = tmppool.tile([P, TC], F32, tag="t2")
                            nc.vector.tensor_mul(t2[:], rot[:], sn_t[:])
                            qk = opool.tile([P, TC], DT, tag="qk")
                            nc.vector.tensor_add(qk[:], t1[:], t2[:])
                            if ob < HQ:
                                nc.sync.dma_start(
                                    qsp[ob, :, ch * TC:(ch + 1) * TC], qk[:])
                            else:
                                nc.sync.dma_start(
                                    ksp[:, ch * TC:(ch + 1) * TC], qk[:])
                        else:
                            # v: transpose [d, t] -> [t, d] blocks and spill
                            vt = rawpool.tile([P, TC], DT, tag="vt")
                            nc.scalar.copy(vt[:], ps[:])
                            for tb in range(TC // P):
                                tp = tppool.tile([P, P], DT)
                                nc.tensor.transpose(
                                    tp[:], vt[:, tb * P:(tb + 1) * P], ident_sb[:])
                                vo = opool.tile([P, P], DT, tag="vo")
                                nc.vector.tensor_copy(vo[:], tp[:])
                                gtb = ch * (TC // P) + tb
                                nc.sync.dma_start(
                                    vsp[b, gtb % (S // P)], vo[:])

            # ---- P2 + P3: attention, then output projection per batch ---
            with (
                tc.tile_pool(name="p3w", bufs=1) as w3pool,
                tc.tile_pool(name="p2kv", bufs=2) as kvpool,
                tc.tile_pool(name="p2q", bufs=2) as qpool,
                tc.tile_pool(name="p2pt", bufs=1) as ptpool,
                tc.tile_pool(name="p2rec", bufs=2) as recpool,
                tc.tile_pool(name="p2ot", bufs=2) as otpool,
                tc.tile_pool(name="p3o", bufs=3) as o3pool,
                tc.tile_pool(name="p3e", bufs=3) as e3pool,
                tc.tile_pool(name="p2sc", bufs=4, space="PSUM") as scpool,
                tc.tile_pool(name="p2sm", bufs=1, space="PSUM") as smpool,
                tc.tile_pool(name="p2op", bufs=1, space="PSUM") as oppool,
                tc.tile_pool(name="p3ps", bufs=2, space="PSUM") as ps3pool,
            ):
                # Wo shard cached up front; the DMA overlaps batch-0 attention.
                # Loads feeding P2/P3 matmuls go over SWDGE (gpsimd) so they
                # don't queue behind P1's bulk X traffic on the sync ring.
                wo_sb = w3pool.tile([P, HQ, HID], DT, tag="wo")
                nc.gpsimd.dma_start(wo_sb[:], wo3[:])

                for b in range(B):
                    kt_sb = kvpool.tile([HD, S], DT, tag="ktb")
                    nc.gpsimd.dma_start(kt_sb[:], ksp[:, b * S:(b + 1) * S])
                    v_sb = kvpool.tile([P, NKB, HD], DT, tag="vb")
                    nc.gpsimd.dma_start(
                        v_sb[:], vsp[b].rearrange("tb p d -> p tb d"))
                    for h in range(HQ):
                        qt_sb = qpool.tile([HD, S], DT, tag="qt")
                        nc.gpsimd.dma_start(
                            qt_sb[:], qsp[h, :, b * S:(b + 1) * S])
                        for qc in range(NQC):
                            nkb = (qc + 1) * (QC // P) if causal[b] else NKB
                            pt = ptpool.tile([P, NKB, QC], DT, tag="pt")
                            sm_ps = smpool.tile([P, QC], F32)
                            ot_ps = oppool.tile([P, QC], F32)
                            sp_qc, sp_tl, sp_kbs = sup_plan[b]

                            def off(kb):
                                return (max(0, kb * P - qc * QC)
                                        if causal[b] else 0)

                            def emit_score(kb):
                                o = off(kb)
                                w = QC - o
                                sc_ps = scpool.tile([P, QC], F32, tag="sc",
                                                    name=f"sc{kb}")
                                nc.tensor.matmul(
                                    sc_ps[:, :w],
                                    kt_sb[:, kb * P:(kb + 1) * P],
                                    qt_sb[:, qc * QC + o:(qc + 1) * QC],
                                    start=True, stop=True)
                                nc.scalar.activation(
                                    pt[:, kb, o:QC], sc_ps[:, :w],
                                    mybir.ActivationFunctionType.Exp,
                                    scale=SCALE)
                                if causal[b] and kb * P >= qc * QC:
                                    nc.vector.tensor_mul(
                                        pt[:, kb, o:o + P],
                                        pt[:, kb, o:o + P], tri_sb[:])

                            def emit_consume(kb):
                                o = off(kb)
                                nc.tensor.matmul(
                                    sm_ps[:, o:QC], ones_sb[:],
                                    pt[:, kb, o:QC],
                                    start=(kb == 0), stop=(kb == nkb - 1))
                                if (qc == sp_qc and kb in sp_kbs
                                        and sp_tl >= o):
                                    nc.vector.tensor_mul(
                                        pt[:, kb, sp_tl:sp_tl + 1],
                                        pt[:, kb, sp_tl:sp_tl + 1],
                                        sup_sb[:, b, kb:kb + 1])
                                nc.tensor.matmul(
                                    ot_ps[:, o:QC], v_sb[:, kb],
                                    pt[:, kb, o:QC],
                                    start=(kb == 0), stop=(kb == nkb - 1))

                            # software pipeline: scores run LA blocks ahead
                            # of the sums/V consumers so the PE never waits
                            # on ScalarE's exp.
                            LA = 4
                            for kb in range(nkb):
                                if kb >= LA:
                                    emit_consume(kb - LA)
                                emit_score(kb)
                            for kb in range(max(0, nkb - LA), nkb):
                                emit_consume(kb)

                            rec = recpool.tile([P, QC], F32, tag="rec")
                            nc.vector.reciprocal(rec[:], sm_ps[:])
                            ot_sb = otpool.tile([P, QC], DT, tag="ot")
                            nc.vector.tensor_mul(ot_sb[:], ot_ps[:], rec[:])
                            nc.sync.dma_start(
                                osp[b, h, :, qc * QC:(qc + 1) * QC], ot_sb[:])

                    # P3 for this batch overlaps batch b+1's K/V/Q loads
                    for tb in range(S // P):
                        ot_t = o3pool.tile([P, HQ, P], DT, tag="o3")
                        nc.gpsimd.dma_start(
                            ot_t[:],
                            osp[b, :, :, tb * P:(tb + 1) * P].rearrange(
                                "h d t -> d h t"))
                        for mc in range(HID // MC):
                            ps = ps3pool.tile([P, MC], F32)
                            for jb in range(HQ):
                                nc.tensor.matmul(
                                    ps[:], ot_t[:, jb],
                                    wo_sb[:, jb, mc * MC:(mc + 1) * MC],
                                    start=(jb == 0), stop=(jb == HQ - 1))
                            ev = e3pool.tile([P, MC], F32, tag="ev")
                            nc.vector.tensor_copy(ev[:], ps[:])
                            nc.sync.dma_start(
                                out[b * S + tb * P:b * S + (tb + 1) * P,
                                    mc * MC:(mc + 1) * MC], ev[:])
    nc.compile()
    return nc


_PROG_CACHE = {}

# Set by a test harness to capture HW profiles: TRACE=True makes kernel()
# pass trace=True to run_bass_kernel_spmd and stash the BassKernelResults
# in LAST_RESULTS. The graded path leaves these defaults alone.
TRACE = False
TRACE_KWARGS = {}
LAST_RESULTS = None


def _mask_mode(attention_mask):
    """Classify the additive mask per batch: True=causal, False=all-zero."""
    m = np.asarray(attention_mask)[:, 0]          # [B, S, S]
    modes = []
    ql, kl = np.tril_indices(S)
    qu, ku = np.triu_indices(S, k=1)
    for b in range(B):
        if np.all(m[b] == 0.0):
            modes.append(False)
        elif np.all(m[b][ql, kl] == 0.0) and np.all(m[b][qu, ku] < -1e30):
            modes.append(True)
        else:
            raise NotImplementedError(
                "attention_mask must be causal or all-zero per batch")
    return tuple(modes)


def kernel(hidden_states, Wq, Wk, Wv, Wo, attention_mask, position_ids,
           tgt_pos, subject_positions):
    hidden_states = np.asarray(hidden_states, dtype=np.float32)
    Wq = np.asarray(Wq, dtype=np.float32)
    Wk = np.asarray(Wk, dtype=np.float32)
    Wv = np.asarray(Wv, dtype=np.float32)
    Wo = np.asarray(Wo, dtype=np.float32)
    position_ids = np.asarray(position_ids)
    tgt_pos = np.asarray(tgt_pos)
    subject_positions = np.asarray(subject_positions)

    # ---- host-side constant prep -----------------------------------------
    XT = np.ascontiguousarray(hidden_states.reshape(T, HID).T)

    inv = 1.0 / (THETA ** (np.arange(0, HD, 2, dtype=np.float64) / HD))
    freqs = position_ids.astype(np.float64)[:, :, None] * inv[None, None, :]
    emb = np.concatenate([freqs, freqs], axis=-1)          # [B, S, HD]
    CS = np.ascontiguousarray(np.cos(emb).reshape(T, HD).T.astype(np.float32))
    SN = np.ascontiguousarray(np.sin(emb).reshape(T, HD).T.astype(np.float32))

    R = np.zeros((HD, HD), dtype=np.float32)               # lhsT of rotate_half
    for dout in range(HD // 2):
        R[dout + HD // 2, dout] = -1.0
    for dout in range(HD // 2, HD):
        R[dout - HD // 2, dout] = 1.0

    TRI = np.triu(np.ones((P, P), dtype=np.float32))       # tri[k, q] = k <= q
    IDENT = np.eye(P, dtype=np.float32)
    ONES = np.ones((P, P), dtype=np.float32)

    M = np.ones((B, S), dtype=np.float32)
    for b in range(B):
        np.multiply.at(M[b], subject_positions[b].astype(np.int64), SUPPRESS)
    SUP = np.ascontiguousarray(
        M.reshape(B, NKB, P).transpose(2, 0, 1))           # [P, B, NKB]

    sup_plan = []
    for b in range(B):
        tb = int(tgt_pos[b])
        kbs = tuple(sorted({int(v) // P for v in subject_positions[b]}))
        sup_plan.append((tb // QC, tb % QC, kbs))
    causal = _mask_mode(attention_mask)
    prog_key = (tuple(sup_plan), causal)

    if prog_key not in _PROG_CACHE:
        _PROG_CACHE[prog_key] = _build_program(sup_plan, causal)
    nc = _PROG_CACHE[prog_key]

    in_maps = []
    for i in range(NCORES):
        wq_i = Wq[i * 512:(i + 1) * 512].T                 # [HID, 512]
        wk_i = Wk[i * HD:(i + 1) * HD].T                   # [HID, 128]
        wv_i = Wv[i * HD:(i + 1) * HD].T
        wqkv_i = np.ascontiguousarray(
            np.concatenate([wq_i, wk_i, wv_i], axis=1))    # [HID, 768]
        wo_i = np.ascontiguousarray(Wo[:, i * 512:(i + 1) * 512].T)
        in_maps.append(dict(
            xt=XT, wqkv=wqkv_i, wo=wo_i, cs=CS, sn=SN, rmat=R, tri=TRI,
            ident=IDENT, ones=ONES, sup=SUP))

    global LAST_RESULTS
    kw = dict(TRACE_KWARGS)
    if TRACE:
        kw.setdefault("trace", True)
    res = run_bass_kernel_spmd(nc, in_maps, list(range(NCORES)), **kw)
    LAST_RESULTS = res
    total = res.results[0]["out"].astype(np.float64)
    for i in range(1, NCORES):
        total += res.results[i]["out"]
    return total.astype(np.float32).reshape(B, S, HID)


# revision 20
# speedup vs baseline: 1.0175x; 1.0175x over previous
"""TRN2 Bass kernel for nn_AttentionWrapper (GQA attention + RoPE + causal mask
+ post-softmax suppression), tensor-parallel over heads across 8 NeuronCores.

Sharding: core i owns q-heads 4i..4i+3 and kv-head i (GQA group i), i.e. rows
512i..512(i+1) of Wq, rows 128i..128(i+1) of Wk/Wv, and columns 512i..512(i+1)
of Wo. hidden_states is replicated; each core emits a full-shape partial of
the output projection and the host sums the 8 partials.

Per-core device program (identical SPMD program, per-core weight data):
  P1  QT/KT = W^T-major projections fused with RoPE (rotate-half via a signed
      permutation matmul), V via XT-stationary matmuls + PE transpose; spill
      QT/KT/V to internal DRAM.
  P2  Flash-style attention per (batch, head) in score-transposed layout
      ST[k,q]: exp on ScalarE (no max subtraction - scores are bounded),
      causal handled by variable-width matmuls + a triangular mask on the
      diagonal blocks, row sums via an all-ones stationary matmul (replicated
      across partitions so normalization broadcasts for free), suppression as
      a single-column multiply, OT = V^T P^T accumulated in PSUM.
  P3  Output projection from OT tiles against the Wo column shard; partial
      written as [B*S, HID] float32.

All matmul operands use dt.float32r (full-rate fp32 PE mode).
"""
import math

import numpy as np

import concourse.bass as bass
import concourse.bacc as bacc
import concourse.mybir as mybir
import concourse.tile as tile
from concourse.bass_utils import run_bass_kernel_spmd

B, S, HID = 2, 2048, 4096
NH, NKV, HD = 32, 8, 128
NCORES = 8
HQ = NH // NCORES            # 4 q heads per core
T = B * S                    # 4096 token axis (b*S + s)
THETA = 10000.0
SUPPRESS = 0.1
SCALE = 1.0 / math.sqrt(HD)

P = 128
TC = 512                     # P1 token chunk
NCH = T // TC                # 8
KS = 4                       # k subtiles per K tile (K_TILE = 512)
NKT = HID // (P * KS)        # 8 K tiles over HID
WC = 512 + HD + HD           # 768 fused wqkv columns (4 q heads, k, v)
QC = 512                     # P2 query chunk
NQC = S // QC                # 4 per batch
NKB = S // P                 # 16 key blocks per batch
MC = 512                     # P3 output column chunk
DT = mybir.dt.float32r
F32 = mybir.dt.float32


def _build_program(sup_plan, causal):
    """sup_plan: list over b of (qc_idx, tloc, sorted kb list).
    causal: per-batch bool — True applies the causal structure (variable-width
    score matmuls + triangular diag mask), False runs full attention."""
    nc = bacc.Bacc("TRN2", target_bir_lowering=False, debug=False)

    xt = nc.dram_tensor("xt", [HID, T], DT, kind="ExternalInput")
    wqkv = nc.dram_tensor("wqkv", [HID, WC], DT, kind="ExternalInput")
    wo = nc.dram_tensor("wo", [HQ * HD, HID], DT, kind="ExternalInput")
    cs = nc.dram_tensor("cs", [HD, T], DT, kind="ExternalInput")
    sn = nc.dram_tensor("sn", [HD, T], DT, kind="ExternalInput")
    rmat = nc.dram_tensor("rmat", [HD, HD], DT, kind="ExternalInput")
    tri = nc.dram_tensor("tri", [P, P], DT, kind="ExternalInput")
    ident = nc.dram_tensor("ident", [P, P], DT, kind="ExternalInput")
    ones = nc.dram_tensor("ones", [P, P], DT, kind="ExternalInput")
    sup = nc.dram_tensor("sup", [P, B, NKB], DT, kind="ExternalInput")
    out = nc.dram_tensor("out", [T, HID], F32, kind="ExternalOutput")

    qsp = nc.dram_tensor("qsp", [HQ, HD, T], DT)
    ksp = nc.dram_tensor("ksp", [HD, T], DT)
    vsp = nc.dram_tensor("vsp", [B, S // P, P, HD], DT)
    osp = nc.dram_tensor("osp", [B, HQ, HD, S], DT)

    xt4 = xt.rearrange("(kt ks p) t -> kt p ks t", p=P, ks=KS)
    wqkv4 = wqkv.rearrange("(kt ks p) c -> kt p ks c", p=P, ks=KS)
    wo3 = wo.rearrange("(jb p) m -> p jb m", p=P)

    with tile.TileContext(nc) as tc:
        # ---- constants used across phases -------------------------------
        with tc.tile_pool(name="const", bufs=1) as cpool:
            rmat_sb = cpool.tile([HD, HD], DT, tag="rmat")
            nc.sync.dma_start(rmat_sb[:], rmat[:])
            tri_sb = cpool.tile([P, P], DT, tag="tri")
            nc.sync.dma_start(tri_sb[:], tri[:])
            ident_sb = cpool.tile([P, P], DT, tag="ident")
            nc.sync.dma_start(ident_sb[:], ident[:])
            ones_sb = cpool.tile([P, P], DT, tag="ones")
            nc.sync.dma_start(ones_sb[:], ones[:])
            sup_sb = cpool.tile([P, B, NKB], DT, tag="sup")
            nc.sync.dma_start(sup_sb[:], sup[:])

            # ---- P1: projections + RoPE ---------------------------------
            with (
                tc.tile_pool(name="p1w", bufs=1) as wpool,
                tc.tile_pool(name="p1x", bufs=4) as xpool,
                tc.tile_pool(name="p1cs", bufs=2) as cspool,
                tc.tile_pool(name="p1raw", bufs=3) as rawpool,
                tc.tile_pool(name="p1tmp", bufs=3) as tmppool,
                tc.tile_pool(name="p1out", bufs=3) as opool,
                tc.tile_pool(name="p1ps", bufs=6, space="PSUM") as pspool,
                tc.tile_pool(name="p1rot", bufs=1, space="PSUM") as rotpool,
                tc.tile_pool(name="p1tp", bufs=1, space="PSUM") as tppool,
            ):
                wts = []
                for kt in range(NKT):
                    wt = wpool.tile([P, KS, WC], DT, tag=f"w{kt}")
                    nc.sync.dma_start(wt[:], wqkv4[kt])
                    wts.append(wt)

                for ch in range(NCH):
                    b = ch // (NCH // B)
                    cs_t = cspool.tile([HD, TC], DT, tag="cs")
                    nc.sync.dma_start(cs_t[:], cs[:, ch * TC:(ch + 1) * TC])
                    sn_t = cspool.tile([HD, TC], DT, tag="sn")
                    nc.sync.dma_start(sn_t[:], sn[:, ch * TC:(ch + 1) * TC])

                    # 6 live accumulators (one PSUM bank each); X streamed
                    # through a small rotating pool, one K-tile at a time.
                    pss = [pspool.tile([P, TC], F32, tag="acc", name=f"acc{ch}_{i}")
                           for i in range(6)]
                    for kt in range(NKT):
                        xtile = xpool.tile([P, KS, TC], DT, tag="x")
                        nc.sync.dma_start(
                            xtile[:], xt4[kt, :, :, ch * TC:(ch + 1) * TC])
                        for ks_ in range(KS):
                            for ob in range(6):
                                nc.tensor.matmul(
                                    pss[ob],
                                    wts[kt][:, ks_, ob * P:(ob + 1) * P],
                                    xtile[:, ks_],
                                    start=(kt == 0 and ks_ == 0),
                                    stop=(kt == NKT - 1 and ks_ == KS - 1))

                    for ob in range(6):
                        ps = pss[ob]
                        if ob < 5:
                            # q head 0..3 or k: RoPE then spill [d, t]
                            raw = rawpool.tile([P, TC], DT, tag="raw")
                            nc.scalar.copy(raw[:], ps[:])
                            rot = rotpool.tile([P, TC], F32)
                            nc.tensor.matmul(rot[:], rmat_sb[:], raw[:],
                                             start=True, stop=True)
                            t1 = tmppool.tile([P, TC], F32, tag="t1")
                            nc.vector.tensor_mul(t1[:], raw[:], cs_t[:])
                            t2 = tmppool.tile([P, TC], F32, tag="t2")
                            nc.vector.tensor_mul(t2[:], rot[:], sn_t[:])
                            qk = opool.tile([P, TC], DT, tag="qk")
                            nc.vector.tensor_add(qk[:], t1[:], t2[:])
                            if ob < HQ:
                                nc.sync.dma_start(
                                    qsp[ob, :, ch * TC:(ch + 1) * TC], qk[:])
                            else:
                                nc.sync.dma_start(
                                    ksp[:, ch * TC:(ch + 1) * TC], qk[:])
                        else:
                            # v: transpose [d, t] -> [t, d] blocks and spill
                            vt = rawpool.tile([P, TC], DT, tag="vt")
                            nc.scalar.copy(vt[:], ps[:])
                            for tb in range(TC // P):
                                tp = tppool.tile([P, P], DT)
                                nc.tensor.transpose(
                                    tp[:], vt[:, tb * P:(tb + 1) * P], ident_sb[:])
                                vo = opool.tile([P, P], DT, tag="vo")
                                nc.vector.tensor_copy(vo[:], tp[:])
                                gtb = ch * (TC // P) + tb
                                nc.sync.dma_start(
                                    vsp[b, gtb % (S // P)], vo[:])

            # ---- P2 + P3: attention, then output projection per batch ---
            # Attention is a single software pipeline over every (b, h, qc)
            # triple: the per-key-block sums/suppress/V consumers trail the
            # score+exp producers by LA blocks so the PE never waits on
            # ScalarE's exp, including across qc/h/b boundaries. P3 shares
            # the "op" PSUM tag (its lifetime never overlaps a live ot_ps).
            with (
                tc.tile_pool(name="p3w", bufs=2) as w3pool,
                tc.tile_pool(name="p2kv", bufs=2) as kvpool,
                tc.tile_pool(name="p2q", bufs=2) as qpool,
                tc.tile_pool(name="p2pt", bufs=2) as ptpool,
                tc.tile_pool(name="p2rec", bufs=2) as recpool,
                tc.tile_pool(name="p2ot", bufs=2) as otpool,
                tc.tile_pool(name="p3o", bufs=3) as o3pool,
                tc.tile_pool(name="p3e", bufs=3) as e3pool,
                tc.tile_pool(name="p2sc", bufs=4, space="PSUM") as scpool,
                tc.tile_pool(name="p2sm", bufs=2, space="PSUM") as smpool,
                tc.tile_pool(name="p2op", bufs=2, space="PSUM") as oppool,
            ):
                LA = 4
                pending = []

                def pump(keep):
                    while len(pending) > keep:
                        pending.pop(0)()

                def make_triple(b, h, qc, kt_sb, v_sb, qt_sb):
                    nkb = (qc + 1) * (QC // P) if causal[b] else NKB
                    pt = ptpool.tile([P, NKB, QC], DT, tag="pt",
                                     name=f"pt{b}_{h}_{qc}")
                    sm_ps = smpool.tile([P, QC], F32, tag="sm",
                                        name=f"sm{b}_{h}_{qc}")
                    ot_ps = oppool.tile([P, QC], F32, tag="op",
                                        name=f"op{b}_{h}_{qc}")
                    sp_qc, sp_tl, sp_kbs = sup_plan[b]

                    def off(kb):
                        return max(0, kb * P - qc * QC) if causal[b] else 0

                    def score(kb):
                        o = off(kb)
                        w = QC - o
                        sc_ps = scpool.tile([P, QC], F32, tag="sc",
                                            name=f"sc{b}_{h}_{qc}_{kb}")
                        nc.tensor.matmul(
                            sc_ps[:, :w],
                            kt_sb[:, kb * P:(kb + 1) * P],
                            qt_sb[:, qc * QC + o:(qc + 1) * QC],
                            start=True, stop=True)
                        nc.scalar.activation(
                            pt[:, kb, o:QC], sc_ps[:, :w],
                            mybir.ActivationFunctionType.Exp, scale=SCALE)
                        if causal[b] and kb * P >= qc * QC:
                            nc.vector.tensor_mul(
                                pt[:, kb, o:o + P], pt[:, kb, o:o + P],
                                tri_sb[:])

                    def consume(kb):
                        def _go():
                            o = off(kb)
                            nc.tensor.matmul(
                                sm_ps[:, o:QC], ones_sb[:], pt[:, kb, o:QC],
                                start=(kb == 0), stop=(kb == nkb - 1))
                            if qc == sp_qc and kb in sp_kbs and sp_tl >= o:
                                nc.vector.tensor_mul(
                                    pt[:, kb, sp_tl:sp_tl + 1],
                                    pt[:, kb, sp_tl:sp_tl + 1],
                                    sup_sb[:, b, kb:kb + 1])
                            nc.tensor.matmul(
                                ot_ps[:, o:QC], v_sb[:, kb], pt[:, kb, o:QC],
                                start=(kb == 0), stop=(kb == nkb - 1))
                        return _go

                    def finalize():
                        rec = recpool.tile([P, QC], F32, tag="rec",
                                           name=f"rec{b}_{h}_{qc}")
                        nc.vector.reciprocal(rec[:], sm_ps[:])
                        ot_sb = otpool.tile([P, QC], DT, tag="ot",
                                            name=f"ot{b}_{h}_{qc}")
                        nc.vector.tensor_mul(ot_sb[:], ot_ps[:], rec[:])
                        nc.sync.dma_start(
                            osp[b, h, :, qc * QC:(qc + 1) * QC], ot_sb[:])

                    return nkb, score, consume, finalize

                for b in range(B):
                    kt_sb = kvpool.tile([HD, S], DT, tag="ktb",
                                        name=f"ktb{b}")
                    nc.gpsimd.dma_start(kt_sb[:], ksp[:, b * S:(b + 1) * S])
                    v_sb = kvpool.tile([P, NKB, HD], DT, tag="vb",
                                       name=f"vb{b}")
                    nc.gpsimd.dma_start(
                        v_sb[:], vsp[b].rearrange("tb p d -> p tb d"))
                    for h in range(HQ):
                        qt_sb = qpool.tile([HD, S], DT, tag="qt",
                                           name=f"qt{b}_{h}")
                        nc.gpsimd.dma_start(
                            qt_sb[:], qsp[h, :, b * S:(b + 1) * S])
                        for qc in range(NQC):
                            nkb, score, consume, finalize = make_triple(
                                b, h, qc, kt_sb, v_sb, qt_sb)
                            for kb in range(nkb):
                                pump(LA - 1)
                                score(kb)
                                pending.append(consume(kb))
                            pending.append(finalize)
                    pump(0)   # drain before this batch's output projection

                    # P3 for this batch; Wo streamed in quarters (double-
                    # buffered) to bound SBUF
                    WQ = HID // 4
                    for quar in range(4):
                        wo_sb = w3pool.tile([P, HQ, WQ], DT, tag="wo",
                                            name=f"wo{b}_{quar}")
                        nc.gpsimd.dma_start(
                            wo_sb[:], wo3[:, :, quar * WQ:(quar + 1) * WQ])
                        for tb in range(S // P):
                            ot_t = o3pool.tile([P, HQ, P], DT, tag="o3",
                                               name=f"o3{b}_{quar}_{tb}")
                            nc.gpsimd.dma_start(
                                ot_t[:],
                                osp[b, :, :, tb * P:(tb + 1) * P].rearrange(
                                    "h d t -> d h t"))
                            for mc in range(WQ // MC):
                                ps = oppool.tile([P, MC], F32, tag="op",
                                                 name=f"p3ps{b}_{quar}_{tb}_{mc}")
                                for jb in range(HQ):
                                    nc.tensor.matmul(
                                        ps[:], ot_t[:, jb],
                                        wo_sb[:, jb, mc * MC:(mc + 1) * MC],
                                        start=(jb == 0), stop=(jb == HQ - 1))
                                ev = e3pool.tile([P, MC], F32, tag="ev",
                                                 name=f"ev{b}_{quar}_{tb}_{mc}")
                                nc.vector.tensor_copy(ev[:], ps[:])
                                moff = quar * WQ + mc * MC
                                nc.sync.dma_start(
                                    out[b * S + tb * P:b * S + (tb + 1) * P,
                                        moff:moff + MC], ev[:])
    nc.compile()
    return nc


_PROG_CACHE = {}

# Set by a test harness to capture HW profiles: TRACE=True makes kernel()
# pass trace=True to run_bass_kernel_spmd and stash the BassKernelResults
# in LAST_RESULTS. The graded path leaves these defaults alone.
TRACE = False
TRACE_KWARGS = {}
LAST_RESULTS = None


def _mask_mode(attention_mask):
    """Classify the additive mask per batch: True=causal, False=all-zero."""
    m = np.asarray(attention_mask)[:, 0]          # [B, S, S]
    modes = []
    ql, kl = np.tril_indices(S)
    qu, ku = np.triu_indices(S, k=1)
    for b in range(B):
        if np.all(m[b] == 0.0):
            modes.append(False)
        elif np.all(m[b][ql, kl] == 0.0) and np.all(m[b][qu, ku] < -1e30):
            modes.append(True)
        else:
            raise NotImplementedError(
                "attention_mask must be causal or all-zero per batch")
    return tuple(modes)


def kernel(hidden_states, Wq, Wk, Wv, Wo, attention_mask, position_ids,
           tgt_pos, subject_positions):
    hidden_states = np.asarray(hidden_states, dtype=np.float32)
    Wq = np.asarray(Wq, dtype=np.float32)
    Wk = np.asarray(Wk, dtype=np.float32)
    Wv = np.asarray(Wv, dtype=np.float32)
    Wo = np.asarray(Wo, dtype=np.float32)
    position_ids = np.asarray(position_ids)
    tgt_pos = np.asarray(tgt_pos)
    subject_positions = np.asarray(subject_positions)

    # ---- host-side constant prep -----------------------------------------
    XT = np.ascontiguousarray(hidden_states.reshape(T, HID).T)

    inv = 1.0 / (THETA ** (np.arange(0, HD, 2, dtype=np.float64) / HD))
    freqs = position_ids.astype(np.float64)[:, :, None] * inv[None, None, :]
    emb = np.concatenate([freqs, freqs], axis=-1)          # [B, S, HD]
    CS = np.ascontiguousarray(np.cos(emb).reshape(T, HD).T.astype(np.float32))
    SN = np.ascontiguousarray(np.sin(emb).reshape(T, HD).T.astype(np.float32))

    R = np.zeros((HD, HD), dtype=np.float32)               # lhsT of rotate_half
    for dout in range(HD // 2):
        R[dout + HD // 2, dout] = -1.0
    for dout in range(HD // 2, HD):
        R[dout - HD // 2, dout] = 1.0

    TRI = np.triu(np.ones((P, P), dtype=np.float32))       # tri[k, q] = k <= q
    IDENT = np.eye(P, dtype=np.float32)
    ONES = np.ones((P, P), dtype=np.float32)

    M = np.ones((B, S), dtype=np.float32)
    for b in range(B):
        np.multiply.at(M[b], subject_positions[b].astype(np.int64), SUPPRESS)
    SUP = np.ascontiguousarray(
        M.reshape(B, NKB, P).transpose(2, 0, 1))           # [P, B, NKB]

    sup_plan = []
    for b in range(B):
        tb = int(tgt_pos[b])
        kbs = tuple(sorted({int(v) // P for v in subject_positions[b]}))
        sup_plan.append((tb // QC, tb % QC, kbs))
    causal = _mask_mode(attention_mask)
    prog_key = (tuple(sup_plan), causal)

    if prog_key not in _PROG_CACHE:
        _PROG_CACHE[prog_key] = _build_program(sup_plan, causal)
    nc = _PROG_CACHE[prog_key]

    in_maps = []
    for i in range(NCORES):
        wq_i = Wq[i * 512:(i + 1) * 512].T                 # [HID, 512]
        wk_i = Wk[i * HD:(i + 1) * HD].T                   # [HID, 128]
        wv_i = Wv[i * HD:(i + 1) * HD].T
        wqkv_i = np.ascontiguousarray(
            np.concatenate([wq_i, wk_i, wv_i], axis=1))    # [HID, 768]
        wo_i = np.ascontiguousarray(Wo[:, i * 512:(i + 1) * 512].T)
        in_maps.append(dict(
            xt=XT, wqkv=wqkv_i, wo=wo_i, cs=CS, sn=SN, rmat=R, tri=TRI,
            ident=IDENT, ones=ONES, sup=SUP))

    global LAST_RESULTS
    kw = dict(TRACE_KWARGS)
    if TRACE:
        kw.setdefault("trace", True)
    res = run_bass_kernel_spmd(nc, in_maps, list(range(NCORES)), **kw)
    LAST_RESULTS = res
    total = res.results[0]["out"].astype(np.float64)
    for i in range(1, NCORES):
        total += res.results[i]["out"]
    return total.astype(np.float32).reshape(B, S, HID)
